# revision 31
# baseline (speedup 1.0000x reference)
"""Trainium2 Bass kernel for a 1D-CNN value network (dense_cnn).

Data-parallel over 8 NeuronCores: batch 32768 -> 4096/core.

Per-core design:
  - Residual stream kept in "TC" layout tiles [128 positions, 128 channels]
    so LayerNorm stats are free-dim reductions (DVE) and the normalize is a
    per-partition-scalar op.
  - Convs run in "CT" layout [128 ch, positions] as weight-stationary
    float32r matmuls (full PE rate at N>=256), 3 shifted taps accumulating
    in PSUM over a per-sample zero-padded SBUF buffer.
  - PE transposes (matmul transpose mode vs a 128x128 identity) bridge
    TC <-> CT.  LN affine + ReLU are fused into the PSUM->SBUF eviction on
    the Scalar engine (per-partition scale/bias = per-channel in CT).
"""

import numpy as np
from contextlib import ExitStack

import concourse.bass as bass
import concourse.bacc as bacc
import concourse.tile as tile
from concourse import mybir
from concourse.bass_utils import run_bass_kernel_spmd
from concourse.masks import make_identity

F32 = mybir.dt.float32
F32R = mybir.dt.float32r
AF = mybir.ActivationFunctionType
OP = mybir.AluOpType

B, L, CIN, F, NBLK = 32768, 24, 15, 128, 9
NCORES = 8
BC = B // NCORES          # 4096 samples per core
S = 16                    # samples per chunk
NCH = BC // S             # 64 chunks
NPOS = S * L              # 1536 positions per chunk
NT = NPOS // 128          # 12 TC tiles per chunk
SSUB = 16                 # samples per conv matmul
NSUB = S // SSUB          # 4 matmul sub-chunks
NSP = SSUB * L            # 384 = conv matmul free size
EPS = 1e-6


def r(ap):
    return ap.bitcast(F32R)


def build(repeat=1):
    nc = bacc.Bacc("TRN2", target_bir_lowering=False, debug=False, num_devices=1)

    d_board = nc.dram_tensor("board_state", [BC, L, CIN], F32, kind="ExternalInput").ap()
    d_aux = nc.dram_tensor("aux_features", [BC, 6], F32, kind="ExternalInput").ap()
    d_c0w = nc.dram_tensor("conv0_w", [7, CIN, F], F32, kind="ExternalInput").ap()
    d_c0b = nc.dram_tensor("conv0_b", [F], F32, kind="ExternalInput").ap()
    d_l1s = nc.dram_tensor("res_ln1_s", [NBLK, F], F32, kind="ExternalInput").ap()
    d_l1b = nc.dram_tensor("res_ln1_b", [NBLK, F], F32, kind="ExternalInput").ap()
    d_w1 = nc.dram_tensor("res_conv1_w", [NBLK, 3, F, F], F32, kind="ExternalInput").ap()
    d_b1 = nc.dram_tensor("res_conv1_b", [NBLK, F], F32, kind="ExternalInput").ap()
    d_l2s = nc.dram_tensor("res_ln2_s", [NBLK, F], F32, kind="ExternalInput").ap()
    d_l2b = nc.dram_tensor("res_ln2_b", [NBLK, F], F32, kind="ExternalInput").ap()
    d_w2 = nc.dram_tensor("res_conv2_w", [NBLK, 3, F, F], F32, kind="ExternalInput").ap()
    d_b2 = nc.dram_tensor("res_conv2_b", [NBLK, F], F32, kind="ExternalInput").ap()
    d_dw = nc.dram_tensor("dense_w", [F + 6, 64], F32, kind="ExternalInput").ap()
    d_db = nc.dram_tensor("dense_b", [64], F32, kind="ExternalInput").ap()
    d_ow = nc.dram_tensor("out_w", [64, 1], F32, kind="ExternalInput").ap()
    d_ob = nc.dram_tensor("out_b", [1], F32, kind="ExternalInput").ap()
    d_out = nc.dram_tensor("out", [BC, 1], F32, kind="ExternalOutput").ap()

    with tile.TileContext(nc) as tc, ExitStack() as ctx:
        P = ctx.enter_context(tc.tile_pool(name="persist", bufs=1))
        WP = ctx.enter_context(tc.tile_pool(name="wts", bufs=1))
        SB = ctx.enter_context(tc.tile_pool(name="work", bufs=5))
        BD = ctx.enter_context(tc.tile_pool(name="board", bufs=3))
        ST = ctx.enter_context(tc.tile_pool(name="stats", bufs=8))
        PS_TR = ctx.enter_context(tc.tile_pool(name="ps_tr", bufs=5, space="PSUM"))
        PS_MM = ctx.enter_context(tc.tile_pool(name="ps_mm", bufs=3, space="PSUM"))

        # ---- weights / constants to SBUF ----
        w0 = WP.tile([CIN, 7, F], F32R, tag="w0")
        w1 = WP.tile([F, NBLK, 3, F], F32R, tag="w1")
        w2 = WP.tile([F, NBLK, 3, F], F32R, tag="w2")
        wst = WP.tile([F, 7, F], F32, tag="wst", bufs=2)
        nc.sync.dma_start(wst[0:CIN, :, :], d_c0w.transpose([1, 0, 2]))
        nc.vector.tensor_copy(w0[:], wst[0:CIN, :, :])
        for blk in range(NBLK):
            wst1 = WP.tile([F, 7, F], F32, tag="wst", bufs=2)
            nc.sync.dma_start(wst1[:, 0:3, :], d_w1[blk].transpose([1, 0, 2]))
            nc.vector.tensor_copy(w1[:, blk, :, :], wst1[:, 0:3, :])
            wst2 = WP.tile([F, 7, F], F32, tag="wst", bufs=2)
            nc.sync.dma_start(wst2[:, 0:3, :], d_w2[blk].transpose([1, 0, 2]))
            nc.vector.tensor_copy(w2[:, blk, :, :], wst2[:, 0:3, :])

        def load_cvec(dram, tag, n=NBLK):  # [n,128] -> sbuf [128, n]
            t = WP.tile([F, n], F32, tag=tag)
            nc.sync.dma_start(t[:], dram.transpose([1, 0]))
            return t

        l1s = load_cvec(d_l1s, "l1s")
        l1b = load_cvec(d_l1b, "l1b")
        l2s = load_cvec(d_l2s, "l2s")
        l2b = load_cvec(d_l2b, "l2b")
        c1b = load_cvec(d_b1, "c1b")
        c2b = load_cvec(d_b2, "c2b")
        c0b = WP.tile([F, 1], F32, tag="c0b")
        nc.sync.dma_start(c0b[:], d_c0b.unsqueeze(-1))

        dwa = WP.tile([F, 64], F32, tag="dwa")
        nc.sync.dma_start(dwa[:], d_dw[0:F, :])
        # fold the 1/24 mean-pool into the dense weights (we pool with sum)
        nc.vector.tensor_scalar(dwa[:], dwa[:], 1.0 / L, None, OP.mult)
        dwb = WP.tile([6, 64], F32, tag="dwb")
        nc.sync.dma_start(dwb[:], d_dw[F:F + 6, :])
        dbv = WP.tile([64, 1], F32, tag="dbv")
        nc.sync.dma_start(dbv[:], d_db.unsqueeze(-1))
        owv = WP.tile([64, 1], F32, tag="owv")
        nc.sync.dma_start(owv[:], d_ow)
        obv = WP.tile([1, 1], F32, tag="obv")
        nc.sync.dma_start(obv[:], d_ob.unsqueeze(-1))

        aux_ct = P.tile([6, BC], F32, tag="auxct")
        nc.sync.dma_start(aux_ct[:], d_aux.transpose([1, 0]))

        epst = WP.tile([128, 1], F32, tag="epst")
        nc.vector.memset(epst[:], EPS)
        ident = WP.tile([128, 128], F32, tag="ident")
        make_identity(nc, ident[:])
        identr = WP.tile([128, 128], F32R, tag="identr")
        nc.vector.tensor_copy(identr[:], ident[:])

        pooled = P.tile([F, BC], F32, tag="pooled")
        stage = P.tile([1, BC], F32, tag="stage")

        # padded conv-input buffers (ping-pong over chunk parity); borders
        # stay zero forever, only valid columns are rewritten
        h1p = [P.tile([F, S, 26], F32R, tag=f"h1p{i}", name=f"h1p{i}") for i in range(5)]
        h2p = [P.tile([F, S, 26], F32R, tag=f"h2p{i}", name=f"h2p{i}") for i in range(5)]
        x0p = [P.tile([CIN, S, 30], F32R, tag=f"x0p{i}", name=f"x0p{i}") for i in range(5)]
        for t in (*h1p, *h2p, *x0p):
            nc.vector.memset(t[:].bitcast(F32), 0.0)

        board_rows = d_board.rearrange("b l c -> (b l) c")

        def transposes(dst_ps, src_sb):
            # PE transpose tiles: TC<->CT bridge (f32r: 1.5 cyc/row)
            for t in range(NT):
                nc.tensor.transpose(
                    dst_ps[:, t * 128:(t + 1) * 128],
                    src_sb[:, t * 128:(t + 1) * 128],
                    identr[:],
                )

        def stats_from(src_view, tag):
            """src_view: [128, NT, 128] (TC). Returns (mv [128,NT,2], rstd [128,NT])."""
            bns = ST.tile([128, NT, 6], F32, tag=f"bns{tag}")
            for t in range(NT):
                nc.vector.bn_stats(bns[:, t, :], src_view[:, t, :])
            mv = ST.tile([128, NT, 2], F32, tag=f"mv{tag}")
            for t in range(NT):
                nc.vector.bn_aggr(mv[:, t, :], bns[:, t, :])
            sd = ST.tile([128, NT], F32, tag=f"sd{tag}")
            nc.scalar.activation(sd[:], mv[:, :, 1], AF.Sqrt, bias=epst[:, 0:1])
            rstd = ST.tile([128, NT], F32, tag=f"rstd{tag}")
            nc.vector.reciprocal(rstd[:], sd[:])
            return mv, rstd

        def conv3(dst_ps, src_pad, w_sb, blk):
            # dst_ps [128, NSUB, 512]; src_pad [128, S, 26]
            for sub in range(NSUB):
                for k in range(3):
                    nc.tensor.matmul(
                        dst_ps[:, sub, 0:NSP],
                        w_sb[:, blk, k, :],
                        src_pad[:, sub * SSUB:(sub + 1) * SSUB, k:k + 24],
                        start=(k == 0), stop=(k == 2),
                    )

        def do_conv0(ch):
            pg = ch % 5
            pos0 = ch * NPOS
            bd = []
            for t in range(NT):
                bt = BD.tile([128, CIN], F32, tag="bd", bufs=16)
                nc.sync.dma_start(bt[:], board_rows[pos0 + t * 128: pos0 + (t + 1) * 128, :])
                bd.append(bt)
            x0t = PS_TR.tile([128, NPOS], F32, tag="tr")
            for t in range(NT):
                nc.tensor.transpose(x0t[0:CIN, t * 128:(t + 1) * 128], bd[t][:], ident[:])
            nc.scalar.activation(
                x0p[pg][:, :, 3:27],
                x0t[0:CIN, :].rearrange("p (s c) -> p s c", s=S), AF.Copy)
            c0 = PS_MM.tile([128, NSUB, 512], F32, tag="mm")
            for sub in range(NSUB):
                for k in range(7):
                    nc.tensor.matmul(
                        c0[:, sub, 0:NSP],
                        w0[:, k, :],
                        x0p[pg][:, sub * SSUB:(sub + 1) * SSUB, k:k + 24],
                        start=(k == 0), stop=(k == 6),
                    )
            x1 = SB.tile([128, NPOS], F32R, tag="z")
            nc.scalar.activation(
                x1[:].rearrange("p (a b) -> p a b", a=NSUB),
                c0[:, :, 0:NSP], AF.Relu, bias=c0b[:, 0:1])
            x1t = PS_TR.tile([128, NPOS], F32R, tag="tr")
            transposes(x1t, x1)
            xtc = SB.tile([128, NT, 128], F32, tag="xtc", bufs=10)
            nc.vector.tensor_copy(xtc[:].rearrange("p t c -> p (t c)"), x1t[:])
            return xtc

        def p1_stats1_norm1(st):
            xtc = st["x"]
            mv1, rstd1 = stats_from(xtc[:], "a")
            z1 = SB.tile([128, NPOS], F32R, tag="z")
            for t in range(NT):
                nc.vector.tensor_scalar(
                    z1[:, t * 128:(t + 1) * 128], xtc[:, t, :],
                    mv1[:, t, 0:1], rstd1[:, t:t + 1],
                    OP.subtract, OP.mult)
            st["z1"] = z1

        def p2_conv1(st, blk):
            pg = st["pg"]
            z1t = PS_TR.tile([128, NPOS], F32R, tag="tr")
            transposes(z1t, st["z1"])
            nc.scalar.activation(
                h1p[pg][:, :, 1:25],
                z1t[:].rearrange("p (s c) -> p s c", s=S), AF.Relu,
                bias=l1b[:, blk:blk + 1], scale=l1s[:, blk:blk + 1])
            g = PS_MM.tile([128, NSUB, 512], F32, tag="mm")
            conv3(g, h1p[pg], w1, blk)
            gsb = SB.tile([128, NPOS], F32R, tag="ev")
            nc.scalar.activation(
                gsb[:].rearrange("p (a b) -> p a b", a=NSUB),
                g[:, :, 0:NSP], AF.Identity, bias=c1b[:, blk:blk + 1])
            st["gsb"] = gsb

        def p3_stats2_norm2(st):
            gt = PS_TR.tile([128, NPOS], F32R, tag="tr")
            transposes(gt, st["gsb"])
            mv2, rstd2 = stats_from(gt[:].rearrange("p (t c) -> p t c", t=NT), "b")
            nmr2 = ST.tile([128, NT], F32, tag="nmr2")
            nc.vector.scalar_tensor_tensor(
                nmr2[:], mv2[:, :, 0], -1.0, rstd2[:], OP.mult, OP.mult)
            z2 = SB.tile([128, NPOS], F32R, tag="z")
            for t in range(NT):
                if t == 0:
                    nc.scalar.activation(
                        z2[:, t * 128:(t + 1) * 128],
                        gt[:, t * 128:(t + 1) * 128], AF.Identity,
                        bias=nmr2[:, t:t + 1], scale=rstd2[:, t:t + 1])
                else:
                    nc.vector.tensor_scalar(
                        z2[:, t * 128:(t + 1) * 128],
                        gt[:, t * 128:(t + 1) * 128],
                        mv2[:, t, 0:1], rstd2[:, t:t + 1],
                        OP.subtract, OP.mult)
            st["z2"] = z2

        def p4_conv2_resid(st, blk):
            pg = st["pg"]
            z2t = PS_TR.tile([128, NPOS], F32R, tag="tr")
            transposes(z2t, st["z2"])
            nc.scalar.activation(
                h2p[pg][:, :, 1:25],
                z2t[:].rearrange("p (s c) -> p s c", s=S), AF.Relu,
                bias=l2b[:, blk:blk + 1], scale=l2s[:, blk:blk + 1])
            p2 = PS_MM.tile([128, NSUB, 512], F32, tag="mm")
            conv3(p2, h2p[pg], w2, blk)
            p2b = SB.tile([128, NPOS], F32R, tag="ev")
            nc.scalar.activation(
                p2b[:].rearrange("p (a b) -> p a b", a=NSUB),
                p2[:, :, 0:NSP], AF.Identity, bias=c2b[:, blk:blk + 1])
            p2t = PS_TR.tile([128, NPOS], F32R, tag="tr")
            transposes(p2t, p2b)
            xnew = SB.tile([128, NT, 128], F32, tag="xtc", bufs=10)
            nc.vector.tensor_tensor(
                xnew[:].rearrange("p t c -> p (t c)"),
                st["x"][:].rearrange("p t c -> p (t c)"), p2t[:], OP.add)
            st["x"] = xnew

        def do_pool(ch, xtc):
            x9t = PS_TR.tile([128, NPOS], F32, tag="tr")
            for t in range(NT):
                nc.tensor.transpose(
                    x9t[:, t * 128:(t + 1) * 128], xtc[:, t, :], ident[:])
            nc.vector.tensor_reduce(
                pooled[:, ch * S:(ch + 1) * S],
                x9t[:].rearrange("p (s l) -> p s l", l=L),
                mybir.AxisListType.X, OP.add)

        chunk_seq = [c for _ in range(repeat) for c in range(NCH)]
        W = 5
        for i in range(0, len(chunk_seq), W):
            chs = chunk_seq[i:i + W]
            states = {}
            for c in chs:
                states[c] = {"x": do_conv0(c), "pg": c % 5}
            for blk in range(NBLK):
                for c in chs:
                    p1_stats1_norm1(states[c])
                for c in chs:
                    p2_conv1(states[c], blk)
                for c in chs:
                    p3_stats2_norm2(states[c])
                for c in chs:
                    p4_conv2_resid(states[c], blk)
            for c in chs:
                do_pool(c, states[c]["x"])

        # ---------- head ----------
        for j in range(BC // 512):
            hps = PS_MM.tile([64, 512], F32, tag="mm")
            nc.tensor.matmul(hps[:], dwa[:], pooled[:, j * 512:(j + 1) * 512],
                             start=True, stop=False)
            nc.tensor.matmul(hps[:], dwb[:], aux_ct[:, j * 512:(j + 1) * 512],
                             start=False, stop=True)
            hh = SB.tile([64, 512], F32, tag="hh")
            nc.scalar.activation(hh[:], hps[:], AF.Relu, bias=dbv[:, 0:1])
            ops = PS_MM.tile([1, 512], F32, tag="mm")
            nc.tensor.matmul(ops[:], owv[:], hh[:], start=True, stop=True)
            nc.scalar.activation(stage[0:1, j * 512:(j + 1) * 512], ops[:],
                                 AF.Tanh, bias=obv[:, 0:1])
        nc.vector.tensor_scalar(stage[:], stage[:], 3.0, None, OP.mult)
        nc.sync.dma_start(d_out.rearrange("b o -> (b o)").unsqueeze(0), stage[:])

    nc.compile()
    return nc


_NC = None


def kernel(**inputs):
    global _NC
    if _NC is None:
        _NC = build()
    full = {k: np.ascontiguousarray(v, dtype=np.float32) for k, v in inputs.items()}
    in_maps = []
    for i in range(NCORES):
        m = {}
        for k, v in full.items():
            if k in ("board_state", "aux_features"):
                m[k] = np.ascontiguousarray(v[i * BC:(i + 1) * BC])
            else:
                m[k] = v
        in_maps.append(m)
    res = run_bass_kernel_spmd(_NC, in_maps, core_ids=list(range(NCORES)))
    return np.concatenate([res.results[i]["out"] for i in range(NCORES)], axis=0)


if __name__ == "__main__":
    rng = np.random.default_rng(0)
    ins = {
        "board_state": rng.standard_normal((B, L, CIN), dtype=np.float32),
        "aux_features": rng.standard_normal((B, 6), dtype=np.float32),
        "conv0_w": rng.standard_normal((7, CIN, F), dtype=np.float32) * 0.05,
        "conv0_b": np.zeros((F,), np.float32),
        "res_ln1_s": np.ones((NBLK, F), np.float32),
        "res_ln1_b": np.zeros((NBLK, F), np.float32),
        "res_conv1_w": rng.standard_normal((NBLK, 3, F, F), dtype=np.float32) * 0.05,
        "res_conv1_b": np.zeros((NBLK, F), np.float32),
        "res_ln2_s": np.ones((NBLK, F), np.float32),
        "res_ln2_b": np.zeros((NBLK, F), np.float32),
        "res_conv2_w": rng.standard_normal((NBLK, 3, F, F), dtype=np.float32) * 0.05,
        "res_conv2_b": np.zeros((NBLK, F), np.float32),
        "dense_w": rng.standard_normal((F + 6, 64), dtype=np.float32) * 0.05,
        "dense_b": np.zeros((64,), np.float32),
        "out_w": rng.standard_normal((64, 1), dtype=np.float32) * 0.05,
        "out_b": np.zeros((1,), np.float32),
    }
    out = kernel(**ins)
    print(out.shape, out[:4, 0])
